# revision 46
# baseline (speedup 1.0000x reference)
# LoftQ fused kernel for Trainium2 (Bass/Tile), 8-core data-parallel, fp8.
#
# reference:
#   W_q = (W_int - zero_point) * scale                  [out=4096, in=4096]
#   W   = W_q + (lora_B @ lora_A) * RANK**-0.5
#   y   = einsum('bsd,od->bso', x, W)                   x: [4, 2048, 4096]
#
# Strategy:
#   - Data-parallel: 8192 tokens sharded 1024/core; W replicated.
#   - Decompose y = s*(x @ W_int.T) - s*zp*rowsum(x) + (x @ A.T) @ (sc*B.T)
#     W_int values 0..15 are EXACT in fp8e4m3, so the main GEMM runs as
#     fp8 x fp8 with MatmulPerfMode.DoubleRow (K=256 per instruction,
#     2x bf16 PE throughput; measured 216ns per [256x128x512] matmul).
#   - x is split hi/lo: xhi = f8(x), xlo = f8(16*(x - xhi)). The main GEMM
#     uses xhi only (its error lands on the small quantized term, which
#     is ~2% of output variance; the LoRA term dominates). The LoRA path
#     u = x @ A_aug.T runs as three fp8 DoubleRow chains:
#     xhi@Ahi(e4m3) + xhi@Alo(e5m2) + xlo@(Ahi/16), giving ~bf16 accuracy.
#   - zero point folded in by augmenting A with a ones row (rank 16->17,
#     zero-padded to 32 for the dual-fp8 ldweights stride rule):
#     u[:,16] = rowsum(x); bts row 16 = -zp. A K=32 bf16 tail matmul adds
#     u @ (sc*B.T/s | -zp) into each main PSUM group; eviction scales by s.
#   - PSUM: 2 banks u-phase + 6 banks main loop (2 oc-chunks per group,
#     pipelined across token-tiles). PE measured ~99% busy in-span.
#   - DMA choreography matters: ~7us NEFF preamble, ~0.6us serial setup
#     per dma_start, ~16-28GB/s per queue (descriptor-size dependent).
#     Tensors are split across many dma_starts in consumption order;
#     the first x block is partition-split so the PE can start at ~11us;
#     the first 3 token-tiles' DR chains are emitted before the xlo
#     u-chain so the PE never idles waiting for x; the last tile's
#     writeback is serialized + partition-split to shorten the drain.
#
# Host-side work is limited to sharding/layout packing (transpose + dtype
# packing); all FLOPs (both matmuls, dequant-by-linearity) run on device.

import numpy as np
import ml_dtypes

import concourse.bass as bass
import concourse.mybir as mybir
import concourse.tile as tile
from concourse import bacc
from concourse.bass import ts
from concourse.bass_utils import run_bass_kernel_spmd

P = 128
N_CORES = 8
RANK = 16
# augmented with a ones-row for the zero-point rowsum, zero-padded to 32:
# dual-fp8 ldweights (DoubleRow) requires the k-pair stride % 16 == 0
RA = 32
# the A-hi and 16*(A - A-hi) stacks live side by side as 64 u columns; the
# 1/16 is folded into the lower 32 bts rows (the tail is ap-bound, so the
# wider K is free and chains 1+2 merge into one DoubleRow chain)
RB = 2 * RA
SCALING = RANK ** (-0.5)
BF16 = mybir.dt.bfloat16
F32 = mybir.dt.float32
F8 = mybir.dt.float8e4
F8E5 = mybir.dt.float8e5
OC = 512      # output-feature chunk (one PSUM bank wide)
OCG = 2       # chunks per oc group (W deadline granularity)
NH = 4        # W chunk DMA'd in NH sub-tiles for startup pipelining

DR = mybir.MatmulPerfMode.DoubleRow
COPY = mybir.ActivationFunctionType.Copy


def build_program(nc, T, D, O, scale):
    """Emit the per-core program.

    T: tokens per core, D: in_features, O: out_features.
    Inputs (per core):
      xhi  f8e4 [P, D/P, T]   f8(x) shard, transposed+partition-packed
      xlo  f8e4 [P, D/P, T]   f8(16*(x - xhi))
      w8p  f8e4 [O/OC, P, D/P, OC]  W_int^T chunk-packed (replicated)
      ahi  f8e4 [P, D/P, RA]  f8(A_aug^T) packed (replicated)
      alo  f8e5 [P, D/P, RA]  f8e5(A_aug^T - ahi) packed (replicated)
      a16  f8e4 [P, D/P, RA]  ahi/16 packed (replicated)
      bts  bf16 [RA, O]       [sc*B.T/s ; -zp ; 0-pad] (replicated)
    Output: y bf16 [T, O]  (host casts to f32); y = scale * psum
    """
    DT, TT, NOC = D // P, T // P, O // OC
    ocg = min(OCG, NOC)
    nh = NH
    while nh > 1 and DT % (2 * nh):
        nh //= 2
    assert DT % (2 * nh) == 0 and NOC % ocg == 0
    HDT = DT // nh       # dt per W sub-tile
    NG = NOC // ocg      # oc groups
    UW = min(512, T)     # u-phase moving width

    xhi = nc.dram_tensor("xhi", [P, DT, T], F8, kind="ExternalInput")
    xlo = nc.dram_tensor("xlo", [P, DT, T], F8, kind="ExternalInput")
    w8 = nc.dram_tensor("w8p", [NOC, P, DT, OC], F8, kind="ExternalInput")
    a2 = nc.dram_tensor("a2", [P, DT, RB], F8, kind="ExternalInput")
    a16 = nc.dram_tensor("a16", [P, DT, RA], F8, kind="ExternalInput")
    bts = nc.dram_tensor("bts", [RB, O], BF16, kind="ExternalInput")
    y = nc.dram_tensor("y", [T, O], BF16, kind="ExternalOutput")
    y_ap = y.ap().rearrange("(tt p) o -> p tt o", p=P)

    with tile.TileContext(nc) as tc:
        with (
            tc.tile_pool(name="const", bufs=1) as cpool,
            tc.tile_pool(name="w8pool", bufs=26) as w8pool,
            tc.tile_pool(name="outpool", bufs=4) as outpool,
            tc.tile_pool(name="psum", bufs=6, space="PSUM") as psum,
            tc.tile_pool(name="psum_u", bufs=2, space="PSUM") as psum_u,
        ):
            # First xhi block lands fastest when split across 4 queues by
            # partition range; issue it before everything else so the
            # u-phase's first matmul can fire early.
            xhi_sb = cpool.tile([P, DT, T], F8)
            B0 = min(2, DT)
            for q in range(4):
                nc.sync.dma_start(
                    xhi_sb[ts(q, P // 4), 0:B0], xhi.ap()[ts(q, P // 4), 0:B0]
                )
            a2_sb = cpool.tile([P, DT, RB], F8)
            nc.sync.dma_start(a2_sb[:], a2.ap())
            a16_sb = cpool.tile([P, DT, RA], F8)
            nc.sync.dma_start(a16_sb[:], a16.ap())
            bts_sb = cpool.tile([RB, O], BF16)
            nc.sync.dma_start(bts_sb[:], bts.ap())

            # Per-queue DMA bandwidth is ~1/16 of HBM, so wide tensors are
            # split across many dma_starts (-> many queues) to land fast.
            XB = max(1, DT // 16)  # dt per xhi/xlo DMA block
            pos = B0
            while pos < DT:
                if pos <= 4 and DT > 8:  # early dts: 2-way partition split
                    for q in range(2):
                        nc.sync.dma_start(
                            xhi_sb[ts(q, P // 2), pos : pos + 1],
                            xhi.ap()[ts(q, P // 2), pos : pos + 1],
                        )
                    pos += 1
                else:
                    b = min(XB, DT - pos)
                    nc.sync.dma_start(
                        xhi_sb[:, pos : pos + b], xhi.ap()[:, pos : pos + b]
                    )
                    pos += b

            # W half-tiles, issued in consumption order; the first group's
            # first halves are split finest (needed right after u-phase).
            wh = {}
            w_order = []
            for g in range(NG):
                for h in range(nh):
                    for oc in range(g * ocg, (g + 1) * ocg):
                        nsub = 2 if (g == 0 or h == 0) else 1
                        nsub = min(nsub, HDT)
                        w_order.append((oc, h, nsub))
            for oc, h, nsub in w_order:
                wh[oc, h] = w8pool.tile([P, HDT, OC], F8, tag="w8", name=f"w8_{oc}_{h}")

            def dma_w(oc, h, nsub):
                sub = HDT // nsub
                for q in range(nsub):
                    nc.sync.dma_start(
                        wh[oc, h][:, q * sub : (q + 1) * sub],
                        w8.ap()[oc, :, h * HDT + q * sub : h * HDT + (q + 1) * sub],
                    )

            for oc, h, nsub in w_order[:ocg]:  # group 0 h0: between xhi and xlo
                dma_w(oc, h, nsub)
            xlo_sb = cpool.tile([P, DT, T], F8)
            pos = 0
            while pos < DT:
                b = min(XB, DT - pos)
                nc.sync.dma_start(
                    xlo_sb[:, pos : pos + b], xlo.ap()[:, pos : pos + b]
                )
                pos += b
            for oc, h, nsub in w_order[ocg:]:
                dma_w(oc, h, nsub)

            # u^T = (x @ A_aug^T)^T computed directly transposed, as fp8
            # DoubleRow chains (dt2-paired, 2x bf16 rate):
            #   u = xhi@Ahi.T + xhi@Alo.T + xlo@(Ahi/16).T
            # hb loops are inner so consumption tracks the dt-ordered x DMAs
            ut_sb = cpool.tile([RB, T], BF16)
            pu = [
                psum_u.tile([RB, UW], F32, tag="pu", name=f"pu_{hb}")
                for hb in range(T // UW)
            ]

            def u_mms(at_sb, x_sb, rows, dt2s, start_at=None, stop_at=None):
                for dt2 in dt2s:
                    for hb in range(T // UW):
                        nc.tensor.matmul(
                            pu[hb][:rows],
                            lhsT=at_sb[:, 2 * dt2 : 2 * dt2 + 2],
                            rhs=x_sb[:, 2 * dt2 : 2 * dt2 + 2, ts(hb, UW)],
                            start=(dt2 == start_at),
                            stop=(dt2 == stop_at),
                            perf_mode=DR,
                        )

            # Main loop: fp8 DoubleRow GEMM per (group, token-tile); the
            # lora+zp tail closes each accumulation group.
            HD2 = HDT // 2  # dt2 pairs per half-tile
            iters = [(g, tt) for g in range(NG) for tt in range(TT)]
            ps = {}

            def emit_drs(g, tt, ocs=None):
                ocs = range(g * ocg, (g + 1) * ocg) if ocs is None else ocs
                for oc in ocs:
                    ps[oc, tt] = psum.tile([P, OC], F32, tag="ps", name=f"ps_{oc}_{tt}")
                for dt2 in range(DT // 2):
                    h, l2 = dt2 // HD2, (dt2 % HD2) * 2
                    for oc in ocs:
                        nc.tensor.matmul(
                            ps[oc, tt][:],
                            lhsT=xhi_sb[:, 2 * dt2 : 2 * dt2 + 2, ts(tt, P)],
                            rhs=wh[oc, h][:, l2 : l2 + 2],
                            start=(dt2 == 0),
                            stop=False,
                            perf_mode=DR,
                        )

            def emit_tail_evict(g, tt, nsp, ocs=None):
                ocs = list(range(g * ocg, (g + 1) * ocg)) if ocs is None else ocs
                for oc in ocs:
                    nc.tensor.matmul(
                        ps[oc, tt][:],
                        lhsT=ut_sb[:, ts(tt, P)],
                        rhs=bts_sb[:, ts(oc, OC)],
                        start=False,
                        stop=True,
                    )
                for oc in ocs:
                    ob = outpool.tile([P, OC], BF16, tag="ob", name=f"ob_{oc}_{tt}")
                    # alternate eviction engines so psum banks recycle faster
                    if oc % 2:
                        nc.scalar.activation(ob[:], ps[oc, tt][:], COPY, scale=scale)
                    else:
                        nc.vector.tensor_scalar(
                            ob[:], ps[oc, tt][:], scale, None, mybir.AluOpType.mult
                        )
                    # partition-split the final tiles' writeback across
                    # queues (full-width descriptors) to shorten the drain
                    for q in range(nsp):
                        pr = ts(q, P // nsp)
                        nc.sync.dma_start(y_ap[pr, tt, ts(oc, OC)], ob[pr])

            # u-phase xhi chains, then the first main tiles (which only need
            # xhi+W), then the xlo chain once xlo has landed, then the rest.
            PRE = min(6 // ocg, len(iters))
            # chain12 (xhi, all but the final dt2 pair), then the first main
            # tiles, then chain3 (xlo), then the final full-width chain12
            # matmul closes the accumulation group.
            u_mms(a2_sb, xhi_sb, RB, range(DT // 2 - 1), start_at=0)
            for g, tt in iters[:PRE]:
                emit_drs(g, tt)
            u_mms(a16_sb, xlo_sb, RA, range(DT // 2))
            u_mms(a2_sb, xhi_sb, RB, [DT // 2 - 1], stop_at=DT // 2 - 1)
            for hb in range(T // UW):
                nc.scalar.activation(ut_sb[:, ts(hb, UW)], pu[hb][:], COPY)
            for i, (g, tt) in enumerate(iters[:PRE]):
                emit_tail_evict(g, tt, 1)
            for i, (g, tt) in enumerate(iters[PRE:-1], start=PRE):
                emit_drs(g, tt)
                emit_tail_evict(g, tt, 1)
            # final tile: serialize its oc chunks so the last writeback is
            # one small, queue-parallel DMA
            gL, ttL = iters[-1]
            for j, oc in enumerate(range(gL * ocg, (gL + 1) * ocg)):
                emit_drs(gL, ttL, ocs=[oc])
                emit_tail_evict(gL, ttL, 2 if j < ocg - 1 else 4, ocs=[oc])
    return nc


def _pack_inputs(x, W_int, lora_A, lora_B, scale, zero_point):
    """Host-side shard + layout packing. Returns per-core input maps."""
    F8NP = ml_dtypes.float8_e4m3
    BFNP = ml_dtypes.bfloat16
    BS, S, D = x.shape
    O = W_int.shape[0]
    Tfull = BS * S
    T = Tfull // N_CORES
    DT = D // P
    NOC = O // OC
    s = float(scale)
    zp = float(zero_point)

    def pack_x(v):  # [T, D] -> [P, DT, T]
        return np.ascontiguousarray(v.T.reshape(DT, P, T).transpose(1, 0, 2))

    xf = np.asarray(x, dtype=np.float32).reshape(Tfull, D)
    # [oc, p, dt, j] <- W_int^T[d=dt*P+p, o=oc*OC+j], exact in fp8e4m3
    w8p = np.ascontiguousarray(
        np.asarray(W_int, dtype=np.float32)
        .astype(F8NP)
        .T.reshape(DT, P, NOC, OC)
        .transpose(2, 1, 0, 3)
    )
    A_aug = np.concatenate(
        [
            np.asarray(lora_A, dtype=np.float32),
            np.ones((1, D), np.float32),
            np.zeros((RA - RANK - 1, D), np.float32),
        ],
        axis=0,
    )  # [RA, D]

    def pack_a(v):  # [R, D] -> [P, DT, R]
        R = v.shape[0]
        return np.ascontiguousarray(v.T.reshape(DT, P, R).transpose(1, 0, 2).astype(F8NP))

    A_hi = A_aug.astype(F8NP).astype(np.float32)
    A_lo16 = ((A_aug - A_hi) * 16.0).astype(F8NP).astype(np.float32)
    a2 = pack_a(np.concatenate([A_hi, A_lo16], axis=0))
    a16 = pack_a(A_hi / 16.0)
    bhalf = np.concatenate(
        [
            np.asarray(lora_B, dtype=np.float32).T * (SCALING / s),
            np.full((1, O), -zp, np.float32),
            np.zeros((RA - RANK - 1, O), np.float32),
        ],
        axis=0,
    )  # [RA, O]
    bts = np.ascontiguousarray(
        np.concatenate([bhalf, bhalf / 16.0], axis=0).astype(BFNP)
    )
    in_maps = []
    for c in range(N_CORES):
        xs = xf[c * T : (c + 1) * T]  # [T, D] f32
        xhi8 = xs.astype(F8NP)
        xlo8 = ((xs - xhi8.astype(np.float32)) * 16.0).astype(F8NP)
        in_maps.append(
            {
                "xhi": pack_x(xhi8),
                "xlo": pack_x(xlo8),
                "w8p": w8p,
                "a2": a2,
                "a16": a16,
                "bts": bts,
            }
        )
    return in_maps, T, D, O


def _install_ntff_shim():
    """Provide antenv.axon_hooks (absent in this image) so that
    run_bass_kernel_spmd(trace=True) can capture NTFF profiles via the
    axon .so — mirrors trn_agent_boot.trn_boot's degraded-silently path.
    Only used for our own measurement runs (_trace=True)."""
    import sys as _sys
    import types as _types

    if "antenv.axon_hooks" in _sys.modules:
        return
    try:
        from trn_agent_boot.trn_boot import _ntff_profile_via_ctypes
    except ImportError:
        _sys.path.insert(0, "/root/.axon_site")
        from trn_agent_boot.trn_boot import _ntff_profile_via_ctypes

    hook = _ntff_profile_via_ctypes("/opt/axon/libaxon_pjrt.so")
    mod = _types.ModuleType("antenv.axon_hooks")
    mod._hook = hook
    mod.get_axon_ntff_profile_hook = lambda: mod._hook
    mod.set_axon_ntff_profile_hook = lambda h: setattr(mod, "_hook", h)
    _sys.modules["antenv.axon_hooks"] = mod
    import antenv as _antenv

    _antenv.axon_hooks = mod


def kernel(x, W_int, lora_A, lora_B, scale, zero_point, _trace=False, _tmpdir=None):
    if _trace:
        _install_ntff_shim()
    x = np.asarray(x)
    BS, S, D = x.shape
    s = float(np.asarray(scale))
    zp = float(np.asarray(zero_point))
    in_maps, T, D, O = _pack_inputs(x, W_int, lora_A, lora_B, s, zp)

    nc = bacc.Bacc(
        "TRN2",
        target_bir_lowering=False,
        debug=False,
        num_devices=N_CORES,
    )
    build_program(nc, T, D, O, scale=s)
    nc.compile()

    res = run_bass_kernel_spmd(
        nc,
        in_maps,
        core_ids=list(range(N_CORES)),
        trace=_trace,
        tmpdir=_tmpdir,
        trace_cores=list(range(N_CORES)) if _trace else None,
    )
    y = (
        np.concatenate([np.asarray(r["y"]) for r in res.results], axis=0)
        .astype(np.float32)
        .reshape(BS, S, O)
    )
    if _trace:
        kernel.last_results = res
    return y


if __name__ == "__main__":
    # smoke: build-only for full shapes
    nc = bacc.Bacc("TRN2", target_bir_lowering=False, debug=False, num_devices=8)
    build_program(nc, 1024, 4096, 4096, scale=0.01)
    nc.compile()
    print("build ok; instructions:", sum(len(b.instructions) for b in nc.main_func.blocks))


# revision 47
# speedup vs baseline: 1.0077x; 1.0077x over previous
# LoftQ fused kernel for Trainium2 (Bass/Tile), 8-core data-parallel, fp8.
#
# reference:
#   W_q = (W_int - zero_point) * scale                  [out=4096, in=4096]
#   W   = W_q + (lora_B @ lora_A) * RANK**-0.5
#   y   = einsum('bsd,od->bso', x, W)                   x: [4, 2048, 4096]
#
# Strategy:
#   - Data-parallel: 8192 tokens sharded 1024/core; W replicated.
#   - Decompose y = s*(x @ W_int.T) - s*zp*rowsum(x) + (x @ A.T) @ (sc*B.T)
#     W_int values 0..15 are EXACT in fp8e4m3, so the main GEMM runs as
#     fp8 x fp8 with MatmulPerfMode.DoubleRow (K=256 per instruction,
#     2x bf16 PE throughput; measured 216ns per [256x128x512] matmul).
#   - x is split hi/lo: xhi = f8(x), xlo = f8(16*(x - xhi)). The main GEMM
#     uses xhi only (its error lands on the small quantized term, which
#     is ~2% of output variance; the LoRA term dominates). The LoRA path
#     u = x @ A_aug.T runs as two fp8e4m3 DoubleRow chains: a 64-wide
#     chain xhi @ [Ahi ; 16*(A-Ahi)] (the 1/16 de-scale is folded into 32
#     extra bts rows, free since the tail is ap-bound) plus a 32-wide
#     xlo @ (Ahi/16) correction chain, giving ~bf16 accuracy.
#   - zero point folded in by augmenting A with a ones row (rank 16->17,
#     zero-padded to 32 for the dual-fp8 ldweights stride rule):
#     u[:,16] = rowsum(x); bts row 16 = -zp. A K=64 bf16 tail matmul adds
#     u @ (sc*B.T/s | -zp) into each main PSUM group; eviction scales by s.
#   - PSUM: 2 banks u-phase + 6 banks main loop (2 oc-chunks per group,
#     pipelined across token-tiles). PE measured ~99% busy in-span.
#   - DMA choreography matters: ~7us NEFF preamble, ~0.6us serial setup
#     per dma_start, ~16-28GB/s per queue (descriptor-size dependent).
#     Tensors are split across many dma_starts in consumption order;
#     the first x block is partition-split so the PE can start at ~11us;
#     the first 3 token-tiles' DR chains are emitted before the xlo
#     u-chain so the PE never idles waiting for x; the last tile's
#     writeback is serialized + partition-split to shorten the drain.
#
# Host-side work is limited to sharding/layout packing (transpose + dtype
# packing); all FLOPs (both matmuls, dequant-by-linearity) run on device.

import numpy as np
import ml_dtypes

import concourse.bass as bass
import concourse.mybir as mybir
import concourse.tile as tile
from concourse import bacc
from concourse.bass import ts
from concourse.bass_utils import run_bass_kernel_spmd

P = 128
N_CORES = 8
RANK = 16
# augmented with a ones-row for the zero-point rowsum, zero-padded to 32:
# dual-fp8 ldweights (DoubleRow) requires the k-pair stride % 16 == 0
RA = 32
# the A-hi and 16*(A - A-hi) stacks live side by side as 64 u columns; the
# 1/16 is folded into the lower 32 bts rows (the tail is ap-bound, so the
# wider K is free and chains 1+2 merge into one DoubleRow chain)
RB = 2 * RA
SCALING = RANK ** (-0.5)
BF16 = mybir.dt.bfloat16
F32 = mybir.dt.float32
F8 = mybir.dt.float8e4
F8E5 = mybir.dt.float8e5
OC = 512      # output-feature chunk (one PSUM bank wide)
OCG = 2       # chunks per oc group (W deadline granularity)
NH = 4        # W chunk DMA'd in NH sub-tiles for startup pipelining

DR = mybir.MatmulPerfMode.DoubleRow
COPY = mybir.ActivationFunctionType.Copy


def build_program(nc, T, D, O, scale):
    """Emit the per-core program.

    T: tokens per core, D: in_features, O: out_features.
    Inputs (per core):
      xhi  f8e4 [P, D/P, T]   f8(x) shard, transposed+partition-packed
      xlo  f8e4 [P, D/P, T]   f8(16*(x - xhi))
      w8p  f8e4 [O/OC, P, D/P, OC]  W_int^T chunk-packed (replicated)
      a2   f8e4 [P, D/P, RB]  [f8(A_aug^T) | f8(16*(A_aug^T - hi))]
      a16  f8e4 [P, D/P, RA]  f8(A_aug^T hi)/16 packed (replicated)
      bts  bf16 [RB, O]       [sc*B.T/s ; -zp ; 0-pad ; same/16] (replicated)
    Output: y bf16 [T, O]  (host casts to f32); y = scale * psum
    """
    DT, TT, NOC = D // P, T // P, O // OC
    ocg = min(OCG, NOC)
    nh = NH
    while nh > 1 and DT % (2 * nh):
        nh //= 2
    assert DT % (2 * nh) == 0 and NOC % ocg == 0
    HDT = DT // nh       # dt per W sub-tile
    NG = NOC // ocg      # oc groups
    UW = min(512, T)     # u-phase moving width

    xhi = nc.dram_tensor("xhi", [P, DT, T], F8, kind="ExternalInput")
    xlo = nc.dram_tensor("xlo", [P, DT, T], F8, kind="ExternalInput")
    w8 = nc.dram_tensor("w8p", [NOC, P, DT, OC], F8, kind="ExternalInput")
    a2 = nc.dram_tensor("a2", [P, DT, RB], F8, kind="ExternalInput")
    a16 = nc.dram_tensor("a16", [P, DT, RA], F8, kind="ExternalInput")
    bts = nc.dram_tensor("bts", [RB, O], BF16, kind="ExternalInput")
    y = nc.dram_tensor("y", [T, O], BF16, kind="ExternalOutput")
    y_ap = y.ap().rearrange("(tt p) o -> p tt o", p=P)

    with tile.TileContext(nc) as tc:
        with (
            tc.tile_pool(name="const", bufs=1) as cpool,
            tc.tile_pool(name="w8pool", bufs=26) as w8pool,
            tc.tile_pool(name="outpool", bufs=4) as outpool,
            tc.tile_pool(name="psum", bufs=6, space="PSUM") as psum,
            tc.tile_pool(name="psum_u", bufs=2, space="PSUM") as psum_u,
        ):
            # First xhi block lands fastest when split across 4 queues by
            # partition range; issue it before everything else so the
            # u-phase's first matmul can fire early.
            xhi_sb = cpool.tile([P, DT, T], F8)
            B0 = min(2, DT)
            for q in range(4):
                nc.sync.dma_start(
                    xhi_sb[ts(q, P // 4), 0:B0], xhi.ap()[ts(q, P // 4), 0:B0]
                )
            a2_sb = cpool.tile([P, DT, RB], F8)
            nc.sync.dma_start(a2_sb[:], a2.ap())
            a16_sb = cpool.tile([P, DT, RA], F8)
            nc.sync.dma_start(a16_sb[:], a16.ap())
            bts_sb = cpool.tile([RB, O], BF16)
            nc.sync.dma_start(bts_sb[:], bts.ap())

            # Per-queue DMA bandwidth is ~1/16 of HBM, so wide tensors are
            # split across many dma_starts (-> many queues) to land fast.
            XB = max(1, DT // 16)  # dt per xhi/xlo DMA block
            pos = B0
            while pos < DT:
                b = min(XB if pos > 4 else 1, DT - pos)
                nc.sync.dma_start(
                    xhi_sb[:, pos : pos + b], xhi.ap()[:, pos : pos + b]
                )
                pos += b

            # W half-tiles, issued in consumption order; the first group's
            # first halves are split finest (needed right after u-phase).
            wh = {}
            w_order = []
            for g in range(NG):
                for h in range(nh):
                    for oc in range(g * ocg, (g + 1) * ocg):
                        nsub = 2 if g == 0 else 1
                        nsub = min(nsub, HDT)
                        w_order.append((oc, h, nsub))
            for oc, h, nsub in w_order:
                wh[oc, h] = w8pool.tile([P, HDT, OC], F8, tag="w8", name=f"w8_{oc}_{h}")

            def dma_w(oc, h, nsub):
                sub = HDT // nsub
                for q in range(nsub):
                    nc.sync.dma_start(
                        wh[oc, h][:, q * sub : (q + 1) * sub],
                        w8.ap()[oc, :, h * HDT + q * sub : h * HDT + (q + 1) * sub],
                    )

            for oc, h, nsub in w_order[:ocg]:  # group 0 h0: between xhi and xlo
                dma_w(oc, h, nsub)
            xlo_sb = cpool.tile([P, DT, T], F8)
            pos = 0
            while pos < DT:
                b = min(XB, DT - pos)
                nc.sync.dma_start(
                    xlo_sb[:, pos : pos + b], xlo.ap()[:, pos : pos + b]
                )
                pos += b
            for oc, h, nsub in w_order[ocg:]:
                dma_w(oc, h, nsub)

            # u^T = (x @ A_aug^T)^T computed directly transposed, as fp8
            # DoubleRow chains (dt2-paired, 2x bf16 rate):
            #   u = xhi@Ahi.T + xhi@Alo.T + xlo@(Ahi/16).T
            # hb loops are inner so consumption tracks the dt-ordered x DMAs
            ut_sb = cpool.tile([RB, T], BF16)
            pu = [
                psum_u.tile([RB, UW], F32, tag="pu", name=f"pu_{hb}")
                for hb in range(T // UW)
            ]

            def u_mms(at_sb, x_sb, rows, dt2s, start_at=None, stop_at=None):
                for dt2 in dt2s:
                    for hb in range(T // UW):
                        nc.tensor.matmul(
                            pu[hb][:rows],
                            lhsT=at_sb[:, 2 * dt2 : 2 * dt2 + 2],
                            rhs=x_sb[:, 2 * dt2 : 2 * dt2 + 2, ts(hb, UW)],
                            start=(dt2 == start_at),
                            stop=(dt2 == stop_at),
                            perf_mode=DR,
                        )

            # Main loop: fp8 DoubleRow GEMM per (group, token-tile); the
            # lora+zp tail closes each accumulation group.
            HD2 = HDT // 2  # dt2 pairs per half-tile
            iters = [(g, tt) for g in range(NG) for tt in range(TT)]
            ps = {}

            def emit_drs(g, tt, ocs=None):
                ocs = range(g * ocg, (g + 1) * ocg) if ocs is None else ocs
                for oc in ocs:
                    ps[oc, tt] = psum.tile([P, OC], F32, tag="ps", name=f"ps_{oc}_{tt}")
                for dt2 in range(DT // 2):
                    h, l2 = dt2 // HD2, (dt2 % HD2) * 2
                    for oc in ocs:
                        nc.tensor.matmul(
                            ps[oc, tt][:],
                            lhsT=xhi_sb[:, 2 * dt2 : 2 * dt2 + 2, ts(tt, P)],
                            rhs=wh[oc, h][:, l2 : l2 + 2],
                            start=(dt2 == 0),
                            stop=False,
                            perf_mode=DR,
                        )

            def emit_tail_evict(g, tt, nsp, ocs=None):
                ocs = list(range(g * ocg, (g + 1) * ocg)) if ocs is None else ocs
                for oc in ocs:
                    nc.tensor.matmul(
                        ps[oc, tt][:],
                        lhsT=ut_sb[:, ts(tt, P)],
                        rhs=bts_sb[:, ts(oc, OC)],
                        start=False,
                        stop=True,
                    )
                for oc in ocs:
                    ob = outpool.tile([P, OC], BF16, tag="ob", name=f"ob_{oc}_{tt}")
                    # alternate eviction engines so psum banks recycle faster
                    if oc % 2:
                        nc.scalar.activation(ob[:], ps[oc, tt][:], COPY, scale=scale)
                    else:
                        nc.vector.tensor_scalar(
                            ob[:], ps[oc, tt][:], scale, None, mybir.AluOpType.mult
                        )
                    # partition-split the final tiles' writeback across
                    # queues (full-width descriptors) to shorten the drain
                    for q in range(nsp):
                        pr = ts(q, P // nsp)
                        nc.sync.dma_start(y_ap[pr, tt, ts(oc, OC)], ob[pr])

            # u-phase xhi chains, then the first main tiles (which only need
            # xhi+W), then the xlo chain once xlo has landed, then the rest.
            PRE = min(6 // ocg, len(iters))
            # chain12 (xhi, all but the final dt2 pair), then the first main
            # tiles, then chain3 (xlo), then the final full-width chain12
            # matmul closes the accumulation group.
            u_mms(a2_sb, xhi_sb, RB, range(DT // 2 - 1), start_at=0)
            for g, tt in iters[:PRE]:
                emit_drs(g, tt)
            u_mms(a16_sb, xlo_sb, RA, range(DT // 2))
            u_mms(a2_sb, xhi_sb, RB, [DT // 2 - 1], stop_at=DT // 2 - 1)
            for hb in range(T // UW):
                nc.scalar.activation(ut_sb[:, ts(hb, UW)], pu[hb][:], COPY)
            for i, (g, tt) in enumerate(iters[:PRE]):
                emit_tail_evict(g, tt, 1)
            for i, (g, tt) in enumerate(iters[PRE:-1], start=PRE):
                emit_drs(g, tt)
                emit_tail_evict(g, tt, 1)
            # final tile: serialize its oc chunks so the last writeback is
            # one small, queue-parallel DMA
            gL, ttL = iters[-1]
            for j, oc in enumerate(range(gL * ocg, (gL + 1) * ocg)):
                emit_drs(gL, ttL, ocs=[oc])
                emit_tail_evict(gL, ttL, 2 if j < ocg - 1 else 4, ocs=[oc])
    return nc


def _pack_inputs(x, W_int, lora_A, lora_B, scale, zero_point):
    """Host-side shard + layout packing. Returns per-core input maps."""
    F8NP = ml_dtypes.float8_e4m3
    BFNP = ml_dtypes.bfloat16
    BS, S, D = x.shape
    O = W_int.shape[0]
    Tfull = BS * S
    T = Tfull // N_CORES
    DT = D // P
    NOC = O // OC
    s = float(scale)
    zp = float(zero_point)

    def pack_x(v):  # [T, D] -> [P, DT, T]
        return np.ascontiguousarray(v.T.reshape(DT, P, T).transpose(1, 0, 2))

    xf = np.asarray(x, dtype=np.float32).reshape(Tfull, D)
    # [oc, p, dt, j] <- W_int^T[d=dt*P+p, o=oc*OC+j], exact in fp8e4m3
    w8p = np.ascontiguousarray(
        np.asarray(W_int, dtype=np.float32)
        .astype(F8NP)
        .T.reshape(DT, P, NOC, OC)
        .transpose(2, 1, 0, 3)
    )
    A_aug = np.concatenate(
        [
            np.asarray(lora_A, dtype=np.float32),
            np.ones((1, D), np.float32),
            np.zeros((RA - RANK - 1, D), np.float32),
        ],
        axis=0,
    )  # [RA, D]

    def pack_a(v):  # [R, D] -> [P, DT, R]
        R = v.shape[0]
        return np.ascontiguousarray(v.T.reshape(DT, P, R).transpose(1, 0, 2).astype(F8NP))

    A_hi = A_aug.astype(F8NP).astype(np.float32)
    A_lo16 = ((A_aug - A_hi) * 16.0).astype(F8NP).astype(np.float32)
    a2 = pack_a(np.concatenate([A_hi, A_lo16], axis=0))
    a16 = pack_a(A_hi / 16.0)
    bhalf = np.concatenate(
        [
            np.asarray(lora_B, dtype=np.float32).T * (SCALING / s),
            np.full((1, O), -zp, np.float32),
            np.zeros((RA - RANK - 1, O), np.float32),
        ],
        axis=0,
    )  # [RA, O]
    bts = np.ascontiguousarray(
        np.concatenate([bhalf, bhalf / 16.0], axis=0).astype(BFNP)
    )
    in_maps = []
    for c in range(N_CORES):
        xs = xf[c * T : (c + 1) * T]  # [T, D] f32
        xhi8 = xs.astype(F8NP)
        xlo8 = ((xs - xhi8.astype(np.float32)) * 16.0).astype(F8NP)
        in_maps.append(
            {
                "xhi": pack_x(xhi8),
                "xlo": pack_x(xlo8),
                "w8p": w8p,
                "a2": a2,
                "a16": a16,
                "bts": bts,
            }
        )
    return in_maps, T, D, O


def _install_ntff_shim():
    """Provide antenv.axon_hooks (absent in this image) so that
    run_bass_kernel_spmd(trace=True) can capture NTFF profiles via the
    axon .so — mirrors trn_agent_boot.trn_boot's degraded-silently path.
    Only used for our own measurement runs (_trace=True)."""
    import sys as _sys
    import types as _types

    if "antenv.axon_hooks" in _sys.modules:
        return
    try:
        from trn_agent_boot.trn_boot import _ntff_profile_via_ctypes
    except ImportError:
        _sys.path.insert(0, "/root/.axon_site")
        from trn_agent_boot.trn_boot import _ntff_profile_via_ctypes

    hook = _ntff_profile_via_ctypes("/opt/axon/libaxon_pjrt.so")
    mod = _types.ModuleType("antenv.axon_hooks")
    mod._hook = hook
    mod.get_axon_ntff_profile_hook = lambda: mod._hook
    mod.set_axon_ntff_profile_hook = lambda h: setattr(mod, "_hook", h)
    _sys.modules["antenv.axon_hooks"] = mod
    import antenv as _antenv

    _antenv.axon_hooks = mod


def kernel(x, W_int, lora_A, lora_B, scale, zero_point, _trace=False, _tmpdir=None):
    if _trace:
        _install_ntff_shim()
    x = np.asarray(x)
    BS, S, D = x.shape
    s = float(np.asarray(scale))
    zp = float(np.asarray(zero_point))
    in_maps, T, D, O = _pack_inputs(x, W_int, lora_A, lora_B, s, zp)

    nc = bacc.Bacc(
        "TRN2",
        target_bir_lowering=False,
        debug=False,
        num_devices=N_CORES,
    )
    build_program(nc, T, D, O, scale=s)
    nc.compile()

    res = run_bass_kernel_spmd(
        nc,
        in_maps,
        core_ids=list(range(N_CORES)),
        trace=_trace,
        tmpdir=_tmpdir,
        trace_cores=list(range(N_CORES)) if _trace else None,
    )
    y = (
        np.concatenate([np.asarray(r["y"]) for r in res.results], axis=0)
        .astype(np.float32)
        .reshape(BS, S, O)
    )
    if _trace:
        kernel.last_results = res
    return y


if __name__ == "__main__":
    # smoke: build-only for full shapes
    nc = bacc.Bacc("TRN2", target_bir_lowering=False, debug=False, num_devices=8)
    build_program(nc, 1024, 4096, 4096, scale=0.01)
    nc.compile()
    print("build ok; instructions:", sum(len(b.instructions) for b in nc.main_func.blocks))
